# revision 13
# baseline (speedup 1.0000x reference)
"""Trainium2 Bass kernel for ConstantTimeStrideAttention (CTSA).

Problem (hardcoded): B=2, S=4096, D=1536, H=12 heads, head dim d=128.
Each query s attends to 12 anchors: band offsets {+-1,+-2,+-3} (weight gw0),
{+-5,+-10} (weight gw1), and globals {0, S-1} (weight gw2 each), where
gw = softmax(group_scale).  softmax over the 12 anchor scores with additive
log-weights == multiplicative weights on exp(score).

Sharding: pure data parallel over (B=2) x (4 sequence chunks of 1024 rows)
-> 8 cores, no collectives.  Each core receives a 1056-row extended slice
of x (2 global rows + 14-left halo + 1024 own + 10-right halo + pad),
pre-transposed and cast to bf16 on the host.

Query tiling: 10 tiles of [102, 103x8, 98] queries.  Each tile's key
window is 125 consecutive extended rows + the 2 global keys = 127 <= 128
keys, so scores / AV / transpose are ONE 128-contraction matmul each
(tile 0's window is rows [0,128) which contains the globals natively).
V is stored per-(tile, head) slot in window order (redundant ~18% v-proj
recompute buys partition-aligned PSUM->SBUF copies and kills the old
window+tail matmul split).

On-core pipeline (bf16 on the PE, fp32 accumulation):
  1a) v projection fc0 for slots 0..5 in kt-OUTER order over 6 PSUM
      banks, consuming the x^T stream as per-kt DMA chunks land (xT on
      the sync queue with growing chunk sizes, wv on the scalar queue).
      The proj_ps pool is allocated BEFORE the phase-1a pool so phase 1b
      needs no PSUM-reuse barrier against the phase-1a copies.
  1b) v projection fc0 slots 6..9, fc1, fc2 (slot-outer); global v rows
      replicated into slot partitions 125:127 by small DMAs.  A ones
      column per slot makes the AV matmul also produce the softmax
      denominator (v bias folded into a host-side constant).
  2+3) per head h: q^T/k^T projection matmuls emitted as a "filler"
      stream with the previous head's attention injected between them,
      software-pipelined A(s)/B(s-2)/C(s-4), one small matmul per unit
      so each LDWEIGHTS hides under a 512-wide projection stream:
        A: scores S^T = matmul(K^T window, Q^T) -> exp (ACT) -> mask (DVE)
        B: AV+denominator in one matmul -> reciprocal, normalize (DVE)
        C: transpose via identity matmul -> A^T copy (DVE)
  4) out projection: Y^T = matmul(lhsT=Wo^T, rhs=A^T) -> bf16 store,
     overlapped with the last head's attention via deferred kt=11
     contraction slices.  fp32 bias/const add happens on host.
Host adds (b_v @ Wo^T + out_b) and stitches chunks together.
"""

import numpy as np
import ml_dtypes

import concourse.mybir as mybir
import concourse.tile as tile
from concourse import bacc
from concourse import bass_utils as _bu
from concourse.bass_utils import run_bass_kernel_spmd

del _bu  # (walrus --enable-ldw-opt=true breaks codegen; keep default)

BF16 = mybir.dt.bfloat16
F32 = mybir.dt.float32

B, S, D = 2, 4096, 1536
H, d = 12, 128
N_CORES = 8
CHUNK = 1024          # own rows per core
XROWS = 1056          # extended rows: 2 glob + 14 halo + 1024 + 10 halo + 6 pad
OWN0 = 16             # first own row inside x_ext
NT = 10               # query tiles per core
VS = 129              # V slot width: 128 features + ones col
ALPHA = float(d) ** -0.5

QW = [102] + [103] * 8 + [98]            # queries per tile
QO = [0, 102, 205, 308, 411, 514, 617, 720, 823, 926]  # tile query offsets
W0 = [0, 108, 211, 314, 417, 520, 623, 726, 829, 925]  # window start rows
WR = [128] + [125] * 9                   # window rows taken from x
KW = [128] + [127] * 9                   # key count (window + globals)

_prog_cache = {}


def _build_program():
    if "nc" in _prog_cache:
        return _prog_cache["nc"]

    nc = bacc.Bacc(
        "TRN2", target_bir_lowering=False, debug=False, num_devices=N_CORES)

    # all inputs pre-swizzled on the host into on-chip layouts so every
    # DMA reads contiguous memory
    xT_d = nc.dram_tensor("xT", [128, D // 128, XROWS], BF16,
                          kind="ExternalInput")
    wqk_d = nc.dram_tensor("wqk", [24, 128, D // 128, 128], BF16,
                           kind="ExternalInput")
    wv_d = nc.dram_tensor("wv", [3, 128, D // 128, 512], BF16,
                          kind="ExternalInput")
    wo_d = nc.dram_tensor("wo", [12, 128, D // 128, 128], BF16,
                          kind="ExternalInput")
    qkbias_d = nc.dram_tensor("qkbias", [128, 24], F32, kind="ExternalInput")
    wmask_d = nc.dram_tensor("wmask", [128, 3, 104], BF16, kind="ExternalInput")
    ident_d = nc.dram_tensor("ident", [128, 128], BF16, kind="ExternalInput")
    yT_d = nc.dram_tensor("yT", [D, CHUNK], BF16, kind="ExternalOutput")

    KO = D // 128  # 12 k-tiles along the contraction dim
    ident_fn = mybir.ActivationFunctionType.Identity
    exp_fn = mybir.ActivationFunctionType.Exp

    with tile.TileContext(nc) as tc:
        with (
            tc.tile_pool(name="persist", bufs=1) as persist,
            tc.tile_pool(name="wq", bufs=3) as wqp,
            tc.tile_pool(name="wv", bufs=2) as wvp,
            tc.tile_pool(name="wo", bufs=3) as wop,
            tc.tile_pool(name="work", bufs=4) as work,
            tc.tile_pool(name="yst", bufs=2) as yst,
            tc.tile_pool(name="proj_ps", bufs=2, space="PSUM") as proj_ps,
        ):
            # ---------- persistent SBUF tensors ----------
            xT = persist.tile([128, KO, XROWS], BF16)
            wv0 = wvp.tile([128, KO, 512], BF16, tag="wv")
            # xT chunks on the sync queue (small first, growing), wv0 on
            # the scalar queue: descriptor issue runs in parallel and the
            # first matmul's deps land after ~0.4MB
            for lo, hi in ((0, 1), (1, 2), (2, 4), (4, 6), (6, 8), (8, 10),
                           (10, 12)):
                nc.sync.dma_start(xT[:, lo:hi, :], xT_d[:, lo:hi, :])
            for lo, hi in ((0, 1), (1, 2), (2, 6), (6, 12)):
                nc.scalar.dma_start(wv0[:, lo:hi, :], wv_d[0][:, lo:hi, :])

            qkbias = persist.tile([128, 24], F32)
            nc.gpsimd.dma_start(qkbias[:], qkbias_d[:])
            wmask = persist.tile([128, 3, 104], BF16)
            nc.gpsimd.dma_start(wmask[:], wmask_d[:])
            ident = persist.tile([128, 128], BF16)
            nc.gpsimd.dma_start(ident[:], ident_d[:])

            QT = persist.tile([128, H, CHUNK], BF16)       # Q^T, s in [16,1040)
            KTw = persist.tile([128, H, NT * 128], BF16)   # K^T window slots
            V = persist.tile([128, NT, H, VS], BF16)       # V window slots
            Vglob = persist.tile([2, D], BF16)
            AT = persist.tile([128, H, CHUNK], BF16)       # attention out ^T

            nc.gpsimd.memset(V[:, :, :, 128:129], 1.0)

            def v_copies(fc, t, ps):
                """PSUM -> SBUF copies for one finished v-proj group."""
                psv = ps.rearrange("p (h f) -> p h f", f=128)
                hs = slice(4 * fc, 4 * fc + 4)
                rows = WR[t]
                nc.vector.tensor_copy(V[0:rows, t, hs, 0:128], psv[0:rows])
                if t == 0:
                    nc.vector.tensor_copy(
                        Vglob[:, fc * 512:(fc + 1) * 512], ps[0:2, :])

            # ---------- phase 1a: fc0 v projection slots 0..5, kt-outer --
            with tc.tile_pool(name="ph1", bufs=6, space="PSUM") as ph1:
                pss = [ph1.tile([128, 512], F32, tag="ph1", name=f"ph1_{i}")
                       for i in range(6)]
                # slots 4, 5 join 2 sweeps late (processing old,
                # already-resident chunks) so the PE consumption rate
                # matches the DMA ramp early on
                for sw in range(KO + 2):
                    for t in range(6):
                        kt = sw if t < 4 else sw - 2
                        if not 0 <= kt < KO:
                            continue
                        nc.tensor.matmul(
                            pss[t][0:WR[t], :],
                            xT[:, kt, W0[t]: W0[t] + WR[t]], wv0[:, kt, :],
                            start=(kt == 0), stop=(kt == KO - 1),
                        )
                for t in range(6):
                    v_copies(0, t, pss[t])

            # ---------- phase 1b: fc0 slots 6..9 + fc1, fc2 --------------
            with (
                tc.tile_pool(name="p3_ps", bufs=1, space="PSUM") as p3_ps,
                tc.tile_pool(name="sc_ps", bufs=2, space="PSUM") as sc_ps,
                tc.tile_pool(name="ad_ps", bufs=3, space="PSUM") as ad_ps,
            ):
                for fc in range(3):
                    if fc >= 1:
                        wv = wvp.tile([128, KO, 512], BF16, tag="wv")
                        nc.scalar.dma_start(wv[:], wv_d[fc])
                        ts = range(NT)
                    else:
                        wv = wv0
                        ts = range(6, NT)
                    for t in ts:
                        ps = proj_ps.tile([128, 512], F32, tag="pps")
                        for kt in range(KO):
                            nc.tensor.matmul(
                                ps[0:WR[t], :],
                                xT[:, kt, W0[t]: W0[t] + WR[t]],
                                wv[:, kt, :],
                                start=(kt == 0), stop=(kt == KO - 1),
                            )
                        v_copies(fc, t, ps)
                # replicate global v rows into slot partitions 125:127
                # (partition shift -> DMA)
                vgv = Vglob.rearrange("p (h f) -> p h f", f=128)
                for t in range(1, NT):
                    nc.gpsimd.dma_start(V[125:127, t, :, 0:128], vgv[:])

                # ---------- phase 2+3: interleaved qk proj + attention ----
                def gen_qk(h):
                    """Yields once per projection matmul (48 per head)."""
                    # q section (f-tile h): own rows only, s in [16, 1040)
                    w = wqp.tile([128, KO, 128], BF16, tag="wq")
                    nc.sync.dma_start(w[:], wqk_d[h])
                    for ncl in range(2):
                        ps = proj_ps.tile([128, 512], F32, tag="pps")
                        for kt in range(KO):
                            nc.tensor.matmul(
                                ps[:], w[:, kt, :],
                                xT[:, kt,
                                   OWN0 + ncl * 512: OWN0 + (ncl + 1) * 512],
                                start=(kt == 0), stop=(kt == KO - 1),
                            )
                            yield
                        # QT = ps*alpha + bias*alpha, on ACT (the
                        # host pre-scales the q-section bias by alpha)
                        nc.scalar.activation(
                            QT[:, h, ncl * 512:(ncl + 1) * 512], ps[:],
                            ident_fn, bias=qkbias[:, h:h + 1], scale=ALPHA)
                    # k section (f-tile 12+h): full extended rows, windowed
                    ft = 12 + h
                    w2 = wqp.tile([128, KO, 128], BF16, tag="wq")
                    nc.sync.dma_start(w2[:], wqk_d[ft])
                    ktw = KTw[:, h, :].rearrange("p (t j) -> p t j", j=128)
                    bias = qkbias[:, ft:ft + 1]
                    ps3 = p3_ps.tile([128, 32], F32, tag="p3")
                    for ncl in range(2):
                        ps = proj_ps.tile([128, 512], F32, tag="pps")
                        for kt in range(KO):
                            nc.tensor.matmul(
                                ps[:], w2[:, kt, :],
                                xT[:, kt, ncl * 512:(ncl + 1) * 512],
                                start=(kt == 0), stop=(kt == KO - 1),
                            )
                            if ncl == 1:
                                # keys 1024..1056: same weights — LDWEIGHTS
                                # hides under the 512-wide stream above
                                nc.tensor.matmul(
                                    ps3[:], w2[:, kt, :], xT[:, kt, 1024:1056],
                                    start=(kt == 0), stop=(kt == KO - 1),
                                )
                            yield
                        if ncl == 0:
                            # slot windows sourced from rows [0, 512)
                            nc.scalar.activation(ktw[:, 0, 0:128],
                                                 ps[:, 0:128], ident_fn,
                                                 bias=bias)
                            for t in range(1, 4):
                                nc.scalar.activation(
                                    ktw[:, t, 0:125],
                                    ps[:, W0[t]:W0[t] + 125], ident_fn,
                                    bias=bias)
                            nc.scalar.activation(ktw[:, 4, 0:95],
                                                 ps[:, 417:512], ident_fn,
                                                 bias=bias)
                            # global key columns for slots 1..9
                            nc.scalar.activation(
                                ktw[:, 1:NT, 125:127],
                                ps[:, None, 0:2].to_broadcast(
                                    [128, NT - 1, 2]),
                                ident_fn, bias=bias)
                        else:
                            # slot windows sourced from rows [512, 1024)
                            nc.scalar.activation(ktw[:, 4, 95:125],
                                                 ps[:, 0:30], ident_fn,
                                                 bias=bias)
                            for t in range(5, 9):
                                nc.scalar.activation(
                                    ktw[:, t, 0:125],
                                    ps[:, W0[t] - 512:W0[t] - 387], ident_fn,
                                    bias=bias)
                            nc.scalar.activation(ktw[:, 9, 0:99],
                                                 ps[:, 413:512], ident_fn,
                                                 bias=bias)
                            nc.scalar.activation(ktw[:, 9, 99:125],
                                                 ps3[:, 0:26], ident_fn,
                                                 bias=bias)

                def att_units(h):
                    """Attention for head h: 3 single-matmul units per
                    query tile (A scores+exp+mask, B AV+normalize, C
                    transpose+copy), pipelined A(s)/B(s-2)/C(s-4)."""
                    ktw = KTw[:, h, :].rearrange("p (t j) -> p t j", j=128)
                    pmS, adS, cS = {}, {}, {}

                    def mkA(t):
                        def A():
                            kw, qw, qo = KW[t], QW[t], QO[t]
                            v = 0 if t == 0 else (2 if t == NT - 1 else 1)
                            sc = sc_ps.tile([128, 104], F32, tag="sc")
                            nc.tensor.matmul(sc[0:kw, 0:qw],
                                             ktw[:, t, 0:kw],
                                             QT[:, h, qo:qo + qw],
                                             start=True, stop=True)
                            pe = work.tile([128, 104], BF16, tag="pe")
                            nc.scalar.activation(pe[0:kw, 0:qw],
                                                 sc[0:kw, 0:qw], exp_fn)
                            pm = work.tile([128, 104], BF16, tag="pm")
                            nc.vector.tensor_mul(pm[0:kw, 0:qw],
                                                 pe[0:kw, 0:qw],
                                                 wmask[0:kw, v, 0:qw])
                            pmS[t] = pm
                        return A

                    def mkB(t):
                        def Bu():
                            kw, qw = KW[t], QW[t]
                            pm = pmS.pop(t)
                            ad = ad_ps.tile([128, 260], F32, tag="ad")
                            nc.tensor.matmul(ad[0:qw, 0:VS], pm[0:kw, 0:qw],
                                             V[0:kw, t, h, :],
                                             start=True, stop=True)
                            r = work.tile([128, 1], F32, tag="r")
                            nc.vector.reciprocal(r[0:qw], ad[0:qw, 128:129])
                            a_sb = work.tile([128, 128], BF16, tag="a_sb")
                            nc.vector.tensor_scalar_mul(a_sb[0:qw, :],
                                                        ad[0:qw, 0:128],
                                                        r[0:qw])
                            cS[t] = (ad, a_sb)
                        return Bu

                    def mkC(t):
                        def C():
                            qw, qo = QW[t], QO[t]
                            ad, a_sb = cS.pop(t)
                            # transpose: A^T = a_sb.T @ I
                            nc.tensor.matmul(ad[:, 132:132 + qw],
                                             a_sb[0:qw, :],
                                             ident[0:qw, 0:qw],
                                             start=True, stop=True)
                            nc.vector.tensor_copy(AT[:, h, qo:qo + qw],
                                                  ad[:, 132:132 + qw])
                        return C

                    units = []
                    for s in range(NT + 4):
                        if s < NT:
                            units.append(mkA(s))
                        if 2 <= s < NT + 2:
                            units.append(mkB(s - 2))
                        if s >= 4:
                            units.append(mkC(s - 4))
                    return units

                def drive(filler_gen, units, nf):
                    ui = 0
                    fcount = 0
                    for _ in filler_gen:
                        fcount += 1
                        quota = (fcount * len(units)) // nf
                        while ui < min(quota, len(units)):
                            units[ui]()
                            ui += 1
                    while ui < len(units):
                        units[ui]()
                        ui += 1

                drive(gen_qk(0), [], 48)
                for h in range(1, H):
                    drive(gen_qk(h), att_units(h - 1), 48)

                # ---------- phase 4: out projection, overlapped with the
                # last head's attention.  The kt=11 (head 11) contraction
                # slices of the first two ncl0 groups are deferred until
                # the attention C units have produced AT[:, 11, 0:512]
                # (tiles 0..4 -> unit C(4) at index 20).
                yT_v = yT_d.rearrange("(fo p) s -> p fo s", p=128)

                def store_y(ps, ft, ncl):
                    y = yst.tile([128, 512], BF16, tag="y")
                    nc.scalar.activation(y[:], ps[:], ident_fn)
                    nc.sync.dma_start(
                        yT_v[:, ft, ncl * 512:(ncl + 1) * 512], y[:])

                u11 = att_units(H - 1)
                wo0 = wop.tile([128, KO, 128], BF16, tag="wo")
                nc.sync.dma_start(wo0[:], wo_d[0])
                wo1 = wop.tile([128, KO, 128], BF16, tag="wo")
                nc.sync.dma_start(wo1[:], wo_d[1])
                defer = []

                def gen_out_deferred():
                    for ft, wo in ((0, wo0), (1, wo1)):
                        ps = proj_ps.tile([128, 512], F32, tag="pps")
                        defer.append(ps)
                        for kt in range(KO - 1):
                            nc.tensor.matmul(
                                ps[:], wo[:, kt, :], AT[:, kt, 0:512],
                                start=(kt == 0), stop=False,
                            )
                            yield

                drive(gen_out_deferred(), u11[0:21], 22)
                # C(0..4) have run: AT[:, 11, 0:512] is complete
                for ps, wo, ft in ((defer[0], wo0, 0), (defer[1], wo1, 1)):
                    nc.tensor.matmul(ps[:], wo[:, KO - 1, :],
                                     AT[:, KO - 1, 0:512],
                                     start=False, stop=True)
                    store_y(ps, ft, 0)

                def gen_out_rest():
                    for ft in range(2, 14):
                        # ft 12/13 redo ft 0/1 (their n1 halves; weights
                        # were evicted by the bufs=3 rotation)
                        fte = ft % 12
                        wo = wop.tile([128, KO, 128], BF16, tag="wo")
                        nc.sync.dma_start(wo[:], wo_d[fte])
                        ncls = (0, 1) if ft < 12 else (1,)
                        for ncl in ncls:
                            ps = proj_ps.tile([128, 512], F32, tag="pps")
                            for kt in range(KO):
                                nc.tensor.matmul(
                                    ps[:], wo[:, kt, :],
                                    AT[:, kt, ncl * 512:(ncl + 1) * 512],
                                    start=(kt == 0), stop=(kt == KO - 1),
                                )
                                yield
                            store_y(ps, fte, ncl)

                drive(gen_out_rest(), u11[21:], 9)

    nc.compile()
    _prog_cache["nc"] = nc
    return nc


def _host_prep(x, qkv_w, qkv_b, out_w, out_b, group_scale):
    """Build the per-core input maps (numpy only)."""
    bf16 = ml_dtypes.bfloat16
    g = np.asarray(group_scale, np.float64)
    e = np.exp(g - g.max())
    gw = (e / e.sum()).astype(np.float64)

    KO = D // 128
    wT = qkv_w.astype(np.float32).T              # [D, 3D]
    # q/k sections, f-tile major: [24, 128, KO, 128]
    wqk = np.ascontiguousarray(
        wT[:, :2 * D].reshape(KO, 128, 24, 128).transpose(2, 1, 0, 3)
    ).astype(bf16)
    # v section, 512-wide f-chunk major: [3, 128, KO, 512]
    wv = np.ascontiguousarray(
        wT[:, 2 * D:].reshape(KO, 128, 3, 512).transpose(2, 1, 0, 3)
    ).astype(bf16)
    woT = out_w.astype(np.float32).T             # [D, D]
    wo = np.ascontiguousarray(
        woT.reshape(KO, 128, 12, 128).transpose(2, 1, 0, 3)
    ).astype(bf16)

    qkbias = np.zeros((128, 24), np.float32)
    for ft in range(24):
        qkbias[:, ft] = qkv_b[ft * 128:(ft + 1) * 128].astype(np.float32)
    qkbias[:, :12] *= ALPHA  # q-path bias pre-scaled (ACT computes in*s+b)

    ident = np.eye(128, dtype=bf16)

    band = [(-1, 0), (1, 0), (-2, 0), (2, 0), (-3, 0), (3, 0),
            (-5, 1), (5, 1), (-10, 1), (10, 1)]

    in_maps = []
    for core in range(N_CORES):
        b, chunk = divmod(core, 4)
        c0 = chunk * CHUNK
        xe = np.zeros((XROWS, D), np.float32)
        xe[0] = x[b, 0]
        xe[1] = x[b, S - 1]
        if chunk > 0:
            xe[2:16] = x[b, c0 - 14:c0]
        xe[16:16 + CHUNK] = x[b, c0:c0 + CHUNK]
        if chunk < 3:
            xe[16 + CHUNK:26 + CHUNK] = x[b, c0 + CHUNK:c0 + CHUNK + 10]
        xT = np.ascontiguousarray(
            xe.T.reshape(KO, 128, XROWS).transpose(1, 0, 2)).astype(bf16)

        # banded weight mask per tile variant, [key j, variant, query p]:
        # variants 0/1/2 = tile 0 / middle tiles / tile 9.  Global keys
        # sit at window rows 0,1 for tile 0 and at key columns 125,126
        # for the other tiles.
        wm = np.zeros((128, 3, 104), np.float64)
        for slot, tv in ((0, 0), (1, 5), (2, NT - 1)):
            for p in range(QW[tv]):
                s = c0 + QO[tv] + p
                for off, grp in band:
                    a = min(max(s + off, 0), S - 1)
                    j = (a - c0 + 16) - W0[tv]
                    if 0 <= j < WR[tv]:
                        wm[j, slot, p] += gw[grp]
            if tv == 0:
                wm[0, slot, :] += gw[2]
                wm[1, slot, :] += gw[2]
            else:
                wm[125, slot, :] += gw[2]
                wm[126, slot, :] += gw[2]

        in_maps.append({
            "xT": xT,
            "wqk": wqk,
            "wv": wv,
            "wo": wo,
            "qkbias": qkbias,
            "wmask": wm.astype(bf16),
            "ident": ident,
        })

    y_const = (qkv_b[2 * D:3 * D].astype(np.float64) @
               out_w.astype(np.float64).T + out_b.astype(np.float64)
               ).astype(np.float32)
    return in_maps, y_const


def kernel(x, qkv_w, qkv_b, out_w, out_b, group_scale, _run_kwargs=None):
    x = np.asarray(x)
    in_maps, y_const = _host_prep(
        np.asarray(x, np.float32), np.asarray(qkv_w, np.float32),
        np.asarray(qkv_b, np.float32), np.asarray(out_w, np.float32),
        np.asarray(out_b, np.float32), np.asarray(group_scale, np.float32))
    nc = _build_program()
    kwargs = _run_kwargs or {}
    res = run_bass_kernel_spmd(nc, in_maps, core_ids=list(range(N_CORES)), **kwargs)
    out = np.empty((B, S, D), np.float32)
    for core in range(N_CORES):
        b, chunk = divmod(core, 4)
        r = res.results[core]
        yT = r["yT"] if isinstance(r, dict) else r
        out[b, chunk * CHUNK:(chunk + 1) * CHUNK] = np.asarray(yT, np.float32).T
    out += y_const
    if kwargs.get("trace"):
        kernel.last_exec_time_ns = res.exec_time_ns
    return out


if __name__ == "__main__":
    rng = np.random.default_rng(0)
    x = rng.standard_normal((B, S, D), dtype=np.float32)
    qkv_w = (rng.standard_normal((3 * D, D), dtype=np.float32) / np.sqrt(D))
    qkv_b = rng.standard_normal(3 * D, dtype=np.float32) * 0.01
    out_w = rng.standard_normal((D, D), dtype=np.float32) / np.sqrt(D)
    out_b = rng.standard_normal(D, dtype=np.float32) * 0.01
    gs = rng.standard_normal(3, dtype=np.float32)
    y = kernel(x=x, qkv_w=qkv_w, qkv_b=qkv_b, out_w=out_w, out_b=out_b,
               group_scale=gs)
    print("ok", y.shape, float(np.abs(y).mean()))


# revision 14
# speedup vs baseline: 1.0114x; 1.0114x over previous
"""Trainium2 Bass kernel for ConstantTimeStrideAttention (CTSA).

Problem (hardcoded): B=2, S=4096, D=1536, H=12 heads, head dim d=128.
Each query s attends to 12 anchors: band offsets {+-1,+-2,+-3} (weight gw0),
{+-5,+-10} (weight gw1), and globals {0, S-1} (weight gw2 each), where
gw = softmax(group_scale).  softmax over the 12 anchor scores with additive
log-weights == multiplicative weights on exp(score).

Sharding: pure data parallel over (B=2) x (4 sequence chunks of 1024 rows)
-> 8 cores, no collectives.  Each core receives a 1056-row extended slice
of x (2 global rows + 14-left halo + 1024 own + 10-right halo + pad),
pre-transposed and cast to bf16 on the host.

Query tiling: 10 tiles of [102, 103x8, 98] queries.  Each tile's key
window is 125 consecutive extended rows + the 2 global keys = 127 <= 128
keys, so scores / AV / transpose are ONE 128-contraction matmul each
(tile 0's window is rows [0,128) which contains the globals natively).
V is stored per-(tile, head) slot in window order (redundant ~18% v-proj
recompute buys partition-aligned PSUM->SBUF copies and kills the old
window+tail matmul split).

On-core pipeline (bf16 on the PE, fp32 accumulation):
  1a) v projection fc0 for slots 0..5 in kt-OUTER order over 6 PSUM
      banks, consuming the x^T stream as per-kt DMA chunks land (xT on
      the sync queue with growing chunk sizes, wv on the scalar queue).
      The proj_ps pool is allocated BEFORE the phase-1a pool so phase 1b
      needs no PSUM-reuse barrier against the phase-1a copies.
  1b) v projection fc0 slots 6..9, fc1, fc2 (slot-outer); global v rows
      replicated into slot partitions 125:127 by small DMAs.  A ones
      column per slot makes the AV matmul also produce the softmax
      denominator (v bias folded into a host-side constant).
  2+3) per head h: q^T/k^T projection matmuls emitted as a "filler"
      stream with the previous head's attention injected between them,
      software-pipelined A(s)/B(s-2)/C(s-4), one small matmul per unit
      so each LDWEIGHTS hides under a 512-wide projection stream:
        A: scores S^T = matmul(K^T window, Q^T) -> exp (ACT) -> mask (DVE)
        B: AV+denominator in one matmul -> reciprocal, normalize (DVE)
        C: transpose via identity matmul -> A^T copy (DVE)
  4) out projection: Y^T = matmul(lhsT=Wo^T, rhs=A^T) -> bf16 store,
     overlapped with the last head's attention via deferred kt=11
     contraction slices.  fp32 bias/const add happens on host.
Host adds (b_v @ Wo^T + out_b) and stitches chunks together.
"""

import numpy as np
import ml_dtypes

import concourse.mybir as mybir
import concourse.tile as tile
from concourse import bacc
from concourse import bass_utils as _bu
from concourse.bass_utils import run_bass_kernel_spmd

del _bu  # (walrus --enable-ldw-opt=true breaks codegen; keep default)

BF16 = mybir.dt.bfloat16
F32 = mybir.dt.float32

B, S, D = 2, 4096, 1536
H, d = 12, 128
N_CORES = 8
CHUNK = 1024          # own rows per core
XROWS = 1056          # extended rows: 2 glob + 14 halo + 1024 + 10 halo + 6 pad
OWN0 = 16             # first own row inside x_ext
NT = 10               # query tiles per core
VS = 129              # V slot width: 128 features + ones col
ALPHA = float(d) ** -0.5

QW = [102] + [103] * 8 + [98]            # queries per tile
QO = [0, 102, 205, 308, 411, 514, 617, 720, 823, 926]  # tile query offsets
W0 = [0, 108, 211, 314, 417, 520, 623, 726, 829, 925]  # window start rows
WR = [128] + [125] * 9                   # window rows taken from x
KW = [128] + [127] * 9                   # key count (window + globals)

_prog_cache = {}


def _build_program():
    if "nc" in _prog_cache:
        return _prog_cache["nc"]

    nc = bacc.Bacc(
        "TRN2", target_bir_lowering=False, debug=False, num_devices=N_CORES)

    # all inputs pre-swizzled on the host into on-chip layouts so every
    # DMA reads contiguous memory
    xT_d = nc.dram_tensor("xT", [128, D // 128, XROWS], BF16,
                          kind="ExternalInput")
    wqk_d = nc.dram_tensor("wqk", [24, 128, D // 128, 128], BF16,
                           kind="ExternalInput")
    wv_d = nc.dram_tensor("wv", [3, 128, D // 128, 512], BF16,
                          kind="ExternalInput")
    wo_d = nc.dram_tensor("wo", [12, 128, D // 128, 128], BF16,
                          kind="ExternalInput")
    qkbias_d = nc.dram_tensor("qkbias", [128, 24], F32, kind="ExternalInput")
    wmask_d = nc.dram_tensor("wmask", [128, 3, 104], BF16, kind="ExternalInput")
    ident_d = nc.dram_tensor("ident", [128, 128], BF16, kind="ExternalInput")
    yT_d = nc.dram_tensor("yT", [D, CHUNK], BF16, kind="ExternalOutput")

    KO = D // 128  # 12 k-tiles along the contraction dim
    ident_fn = mybir.ActivationFunctionType.Identity
    exp_fn = mybir.ActivationFunctionType.Exp

    with tile.TileContext(nc) as tc:
        with (
            tc.tile_pool(name="persist", bufs=1) as persist,
            tc.tile_pool(name="wq", bufs=3) as wqp,
            tc.tile_pool(name="wv", bufs=2) as wvp,
            tc.tile_pool(name="wo", bufs=3) as wop,
            tc.tile_pool(name="work", bufs=4) as work,
            tc.tile_pool(name="yst", bufs=2) as yst,
            tc.tile_pool(name="proj_ps", bufs=2, space="PSUM") as proj_ps,
        ):
            # ---------- persistent SBUF tensors ----------
            xT = persist.tile([128, KO, XROWS], BF16)
            wv0 = wvp.tile([128, KO, 512], BF16, tag="wv")
            # xT chunks on the sync queue (small first, growing), wv0 on
            # the scalar queue: descriptor issue runs in parallel and the
            # first matmul's deps land after ~0.4MB
            for lo, hi in ((0, 1), (1, 2), (2, 4), (4, 6), (6, 8), (8, 10),
                           (10, 11), (11, 12)):
                nc.sync.dma_start(xT[:, lo:hi, :], xT_d[:, lo:hi, :])
            for lo, hi in ((0, 1), (1, 2), (2, 6), (6, 9), (9, 11), (11, 12)):
                nc.scalar.dma_start(wv0[:, lo:hi, :], wv_d[0][:, lo:hi, :])

            qkbias = persist.tile([128, 24], F32)
            nc.gpsimd.dma_start(qkbias[:], qkbias_d[:])
            wmask = persist.tile([128, 3, 104], BF16)
            nc.gpsimd.dma_start(wmask[:], wmask_d[:])
            ident = persist.tile([128, 128], BF16)
            nc.gpsimd.dma_start(ident[:], ident_d[:])

            QT = persist.tile([128, H, CHUNK], BF16)       # Q^T, s in [16,1040)
            KTw = persist.tile([128, H, NT * 128], BF16)   # K^T window slots
            V = persist.tile([128, NT, H, VS], BF16)       # V window slots
            Vglob = persist.tile([2, D], BF16)
            AT = persist.tile([128, H, CHUNK], BF16)       # attention out ^T

            nc.gpsimd.memset(V[:, :, :, 128:129], 1.0)

            def v_copies(fc, t, ps):
                """PSUM -> SBUF copies for one finished v-proj group."""
                psv = ps.rearrange("p (h f) -> p h f", f=128)
                hs = slice(4 * fc, 4 * fc + 4)
                rows = WR[t]
                nc.vector.tensor_copy(V[0:rows, t, hs, 0:128], psv[0:rows])
                if t == 0:
                    nc.vector.tensor_copy(
                        Vglob[:, fc * 512:(fc + 1) * 512], ps[0:2, :])

            # ---------- phase 1a: fc0 v projection slots 0..5, kt-outer --
            with tc.tile_pool(name="ph1", bufs=6, space="PSUM") as ph1:
                pss = [ph1.tile([128, 512], F32, tag="ph1", name=f"ph1_{i}")
                       for i in range(6)]
                # slots 4, 5 join 2 sweeps late (processing old,
                # already-resident chunks) so the PE consumption rate
                # matches the DMA ramp early on
                for sw in range(KO + 2):
                    for t in range(6):
                        kt = sw if t < 4 else sw - 2
                        if not 0 <= kt < KO:
                            continue
                        nc.tensor.matmul(
                            pss[t][0:WR[t], :],
                            xT[:, kt, W0[t]: W0[t] + WR[t]], wv0[:, kt, :],
                            start=(kt == 0), stop=(kt == KO - 1),
                        )
                for t in range(6):
                    v_copies(0, t, pss[t])

            # ---------- phase 1b: fc0 slots 6..9 + fc1, fc2 --------------
            with (
                tc.tile_pool(name="p3_ps", bufs=1, space="PSUM") as p3_ps,
                tc.tile_pool(name="sc_ps", bufs=2, space="PSUM") as sc_ps,
                tc.tile_pool(name="ad_ps", bufs=3, space="PSUM") as ad_ps,
            ):
                for fc in range(3):
                    if fc >= 1:
                        wv = wvp.tile([128, KO, 512], BF16, tag="wv")
                        nc.scalar.dma_start(wv[:], wv_d[fc])
                        ts = range(NT)
                    else:
                        wv = wv0
                        ts = range(6, NT)
                    for t in ts:
                        ps = proj_ps.tile([128, 512], F32, tag="pps")
                        for kt in range(KO):
                            nc.tensor.matmul(
                                ps[0:WR[t], :],
                                xT[:, kt, W0[t]: W0[t] + WR[t]],
                                wv[:, kt, :],
                                start=(kt == 0), stop=(kt == KO - 1),
                            )
                        v_copies(fc, t, ps)
                # replicate global v rows into slot partitions 125:127
                # (partition shift -> DMA)
                vgv = Vglob.rearrange("p (h f) -> p h f", f=128)
                for t in range(1, NT):
                    nc.gpsimd.dma_start(V[125:127, t, :, 0:128], vgv[:])

                # ---------- phase 2+3: interleaved qk proj + attention ----
                def gen_qk(h):
                    """Yields once per projection matmul (48 per head)."""
                    # q section (f-tile h): own rows only, s in [16, 1040)
                    w = wqp.tile([128, KO, 128], BF16, tag="wq")
                    nc.sync.dma_start(w[:], wqk_d[h])
                    for ncl in range(2):
                        ps = proj_ps.tile([128, 512], F32, tag="pps")
                        for kt in range(KO):
                            nc.tensor.matmul(
                                ps[:], w[:, kt, :],
                                xT[:, kt,
                                   OWN0 + ncl * 512: OWN0 + (ncl + 1) * 512],
                                start=(kt == 0), stop=(kt == KO - 1),
                            )
                            yield
                        # QT = ps*alpha + bias*alpha, on ACT (the
                        # host pre-scales the q-section bias by alpha)
                        nc.scalar.activation(
                            QT[:, h, ncl * 512:(ncl + 1) * 512], ps[:],
                            ident_fn, bias=qkbias[:, h:h + 1], scale=ALPHA)
                    # k section (f-tile 12+h): full extended rows, windowed
                    ft = 12 + h
                    w2 = wqp.tile([128, KO, 128], BF16, tag="wq")
                    nc.sync.dma_start(w2[:], wqk_d[ft])
                    ktw = KTw[:, h, :].rearrange("p (t j) -> p t j", j=128)
                    bias = qkbias[:, ft:ft + 1]
                    ps3 = p3_ps.tile([128, 32], F32, tag="p3")
                    for ncl in range(2):
                        ps = proj_ps.tile([128, 512], F32, tag="pps")
                        for kt in range(KO):
                            nc.tensor.matmul(
                                ps[:], w2[:, kt, :],
                                xT[:, kt, ncl * 512:(ncl + 1) * 512],
                                start=(kt == 0), stop=(kt == KO - 1),
                            )
                            if ncl == 1:
                                # keys 1024..1056: same weights — LDWEIGHTS
                                # hides under the 512-wide stream above
                                nc.tensor.matmul(
                                    ps3[:], w2[:, kt, :], xT[:, kt, 1024:1056],
                                    start=(kt == 0), stop=(kt == KO - 1),
                                )
                            yield
                        if ncl == 0:
                            # slot windows sourced from rows [0, 512)
                            nc.scalar.activation(ktw[:, 0, 0:128],
                                                 ps[:, 0:128], ident_fn,
                                                 bias=bias)
                            for t in range(1, 4):
                                nc.scalar.activation(
                                    ktw[:, t, 0:125],
                                    ps[:, W0[t]:W0[t] + 125], ident_fn,
                                    bias=bias)
                            nc.scalar.activation(ktw[:, 4, 0:95],
                                                 ps[:, 417:512], ident_fn,
                                                 bias=bias)
                            # global key columns for slots 1..9
                            nc.scalar.activation(
                                ktw[:, 1:NT, 125:127],
                                ps[:, None, 0:2].to_broadcast(
                                    [128, NT - 1, 2]),
                                ident_fn, bias=bias)
                        else:
                            # slot windows sourced from rows [512, 1024)
                            nc.scalar.activation(ktw[:, 4, 95:125],
                                                 ps[:, 0:30], ident_fn,
                                                 bias=bias)
                            for t in range(5, 9):
                                nc.scalar.activation(
                                    ktw[:, t, 0:125],
                                    ps[:, W0[t] - 512:W0[t] - 387], ident_fn,
                                    bias=bias)
                            nc.scalar.activation(ktw[:, 9, 0:99],
                                                 ps[:, 413:512], ident_fn,
                                                 bias=bias)
                            nc.scalar.activation(ktw[:, 9, 99:125],
                                                 ps3[:, 0:26], ident_fn,
                                                 bias=bias)

                def att_units(h):
                    """Attention for head h: 3 single-matmul units per
                    query tile (A scores+exp+mask, B AV+normalize, C
                    transpose+copy), pipelined A(s)/B(s-2)/C(s-4)."""
                    ktw = KTw[:, h, :].rearrange("p (t j) -> p t j", j=128)
                    pmS, adS, cS = {}, {}, {}

                    def mkA(t):
                        def A():
                            kw, qw, qo = KW[t], QW[t], QO[t]
                            v = 0 if t == 0 else (2 if t == NT - 1 else 1)
                            sc = sc_ps.tile([128, 104], F32, tag="sc")
                            nc.tensor.matmul(sc[0:kw, 0:qw],
                                             ktw[:, t, 0:kw],
                                             QT[:, h, qo:qo + qw],
                                             start=True, stop=True)
                            pe = work.tile([128, 104], BF16, tag="pe")
                            nc.scalar.activation(pe[0:kw, 0:qw],
                                                 sc[0:kw, 0:qw], exp_fn)
                            pm = work.tile([128, 104], BF16, tag="pm")
                            nc.vector.tensor_mul(pm[0:kw, 0:qw],
                                                 pe[0:kw, 0:qw],
                                                 wmask[0:kw, v, 0:qw])
                            pmS[t] = pm
                        return A

                    def mkB(t):
                        def Bu():
                            kw, qw = KW[t], QW[t]
                            pm = pmS.pop(t)
                            ad = ad_ps.tile([128, 260], F32, tag="ad")
                            nc.tensor.matmul(ad[0:qw, 0:VS], pm[0:kw, 0:qw],
                                             V[0:kw, t, h, :],
                                             start=True, stop=True)
                            r = work.tile([128, 1], F32, tag="r")
                            nc.vector.reciprocal(r[0:qw], ad[0:qw, 128:129])
                            a_sb = work.tile([128, 128], BF16, tag="a_sb")
                            nc.vector.tensor_scalar_mul(a_sb[0:qw, :],
                                                        ad[0:qw, 0:128],
                                                        r[0:qw])
                            cS[t] = (ad, a_sb)
                        return Bu

                    def mkC(t):
                        def C():
                            qw, qo = QW[t], QO[t]
                            ad, a_sb = cS.pop(t)
                            # transpose: A^T = a_sb.T @ I
                            nc.tensor.matmul(ad[:, 132:132 + qw],
                                             a_sb[0:qw, :],
                                             ident[0:qw, 0:qw],
                                             start=True, stop=True)
                            nc.vector.tensor_copy(AT[:, h, qo:qo + qw],
                                                  ad[:, 132:132 + qw])
                        return C

                    units = []
                    for s in range(NT + 4):
                        if s < NT:
                            units.append(mkA(s))
                        if 2 <= s < NT + 2:
                            units.append(mkB(s - 2))
                        if s >= 4:
                            units.append(mkC(s - 4))
                    return units

                def drive(filler_gen, units, nf):
                    ui = 0
                    fcount = 0
                    for _ in filler_gen:
                        fcount += 1
                        quota = (fcount * len(units)) // nf
                        while ui < min(quota, len(units)):
                            units[ui]()
                            ui += 1
                    while ui < len(units):
                        units[ui]()
                        ui += 1

                drive(gen_qk(0), [], 48)
                for h in range(1, H):
                    drive(gen_qk(h), att_units(h - 1), 48)

                # ---------- phase 4: out projection, overlapped with the
                # last head's attention.  The kt=11 (head 11) contraction
                # slices of the first two ncl0 groups are deferred until
                # the attention C units have produced AT[:, 11, 0:512]
                # (tiles 0..4 -> unit C(4) at index 20).
                yT_v = yT_d.rearrange("(fo p) s -> p fo s", p=128)

                def store_y(ps, ft, ncl):
                    y = yst.tile([128, 512], BF16, tag="y")
                    nc.scalar.activation(y[:], ps[:], ident_fn)
                    nc.sync.dma_start(
                        yT_v[:, ft, ncl * 512:(ncl + 1) * 512], y[:])

                u11 = att_units(H - 1)
                wo0 = wop.tile([128, KO, 128], BF16, tag="wo")
                nc.sync.dma_start(wo0[:], wo_d[0])
                wo1 = wop.tile([128, KO, 128], BF16, tag="wo")
                nc.sync.dma_start(wo1[:], wo_d[1])
                defer = []

                def gen_out_deferred():
                    for ft, wo in ((0, wo0), (1, wo1)):
                        ps = proj_ps.tile([128, 512], F32, tag="pps")
                        defer.append(ps)
                        for kt in range(KO - 1):
                            nc.tensor.matmul(
                                ps[:], wo[:, kt, :], AT[:, kt, 0:512],
                                start=(kt == 0), stop=False,
                            )
                            yield

                drive(gen_out_deferred(), u11[0:21], 22)
                # C(0..4) have run: AT[:, 11, 0:512] is complete
                for ps, wo, ft in ((defer[0], wo0, 0), (defer[1], wo1, 1)):
                    nc.tensor.matmul(ps[:], wo[:, KO - 1, :],
                                     AT[:, KO - 1, 0:512],
                                     start=False, stop=True)
                    store_y(ps, ft, 0)

                def gen_out_rest():
                    for ft in range(2, 14):
                        # ft 12/13 redo ft 0/1 (their n1 halves; weights
                        # were evicted by the bufs=3 rotation)
                        fte = ft % 12
                        wo = wop.tile([128, KO, 128], BF16, tag="wo")
                        nc.sync.dma_start(wo[:], wo_d[fte])
                        ncls = (0, 1) if ft < 12 else (1,)
                        for ncl in ncls:
                            ps = proj_ps.tile([128, 512], F32, tag="pps")
                            for kt in range(KO):
                                nc.tensor.matmul(
                                    ps[:], wo[:, kt, :],
                                    AT[:, kt, ncl * 512:(ncl + 1) * 512],
                                    start=(kt == 0), stop=(kt == KO - 1),
                                )
                                yield
                            store_y(ps, fte, ncl)

                drive(gen_out_rest(), u11[21:], 9)

    nc.compile()
    _prog_cache["nc"] = nc
    return nc


def _host_prep(x, qkv_w, qkv_b, out_w, out_b, group_scale):
    """Build the per-core input maps (numpy only)."""
    bf16 = ml_dtypes.bfloat16
    g = np.asarray(group_scale, np.float64)
    e = np.exp(g - g.max())
    gw = (e / e.sum()).astype(np.float64)

    KO = D // 128
    wT = qkv_w.astype(np.float32).T              # [D, 3D]
    # q/k sections, f-tile major: [24, 128, KO, 128]
    wqk = np.ascontiguousarray(
        wT[:, :2 * D].reshape(KO, 128, 24, 128).transpose(2, 1, 0, 3)
    ).astype(bf16)
    # v section, 512-wide f-chunk major: [3, 128, KO, 512]
    wv = np.ascontiguousarray(
        wT[:, 2 * D:].reshape(KO, 128, 3, 512).transpose(2, 1, 0, 3)
    ).astype(bf16)
    woT = out_w.astype(np.float32).T             # [D, D]
    wo = np.ascontiguousarray(
        woT.reshape(KO, 128, 12, 128).transpose(2, 1, 0, 3)
    ).astype(bf16)

    qkbias = np.zeros((128, 24), np.float32)
    for ft in range(24):
        qkbias[:, ft] = qkv_b[ft * 128:(ft + 1) * 128].astype(np.float32)
    qkbias[:, :12] *= ALPHA  # q-path bias pre-scaled (ACT computes in*s+b)

    ident = np.eye(128, dtype=bf16)

    band = [(-1, 0), (1, 0), (-2, 0), (2, 0), (-3, 0), (3, 0),
            (-5, 1), (5, 1), (-10, 1), (10, 1)]

    in_maps = []
    for core in range(N_CORES):
        b, chunk = divmod(core, 4)
        c0 = chunk * CHUNK
        xe = np.zeros((XROWS, D), np.float32)
        xe[0] = x[b, 0]
        xe[1] = x[b, S - 1]
        if chunk > 0:
            xe[2:16] = x[b, c0 - 14:c0]
        xe[16:16 + CHUNK] = x[b, c0:c0 + CHUNK]
        if chunk < 3:
            xe[16 + CHUNK:26 + CHUNK] = x[b, c0 + CHUNK:c0 + CHUNK + 10]
        xT = np.ascontiguousarray(
            xe.T.reshape(KO, 128, XROWS).transpose(1, 0, 2)).astype(bf16)

        # banded weight mask per tile variant, [key j, variant, query p]:
        # variants 0/1/2 = tile 0 / middle tiles / tile 9.  Global keys
        # sit at window rows 0,1 for tile 0 and at key columns 125,126
        # for the other tiles.
        wm = np.zeros((128, 3, 104), np.float64)
        for slot, tv in ((0, 0), (1, 5), (2, NT - 1)):
            for p in range(QW[tv]):
                s = c0 + QO[tv] + p
                for off, grp in band:
                    a = min(max(s + off, 0), S - 1)
                    j = (a - c0 + 16) - W0[tv]
                    if 0 <= j < WR[tv]:
                        wm[j, slot, p] += gw[grp]
            if tv == 0:
                wm[0, slot, :] += gw[2]
                wm[1, slot, :] += gw[2]
            else:
                wm[125, slot, :] += gw[2]
                wm[126, slot, :] += gw[2]

        in_maps.append({
            "xT": xT,
            "wqk": wqk,
            "wv": wv,
            "wo": wo,
            "qkbias": qkbias,
            "wmask": wm.astype(bf16),
            "ident": ident,
        })

    y_const = (qkv_b[2 * D:3 * D].astype(np.float64) @
               out_w.astype(np.float64).T + out_b.astype(np.float64)
               ).astype(np.float32)
    return in_maps, y_const


def kernel(x, qkv_w, qkv_b, out_w, out_b, group_scale, _run_kwargs=None):
    x = np.asarray(x)
    in_maps, y_const = _host_prep(
        np.asarray(x, np.float32), np.asarray(qkv_w, np.float32),
        np.asarray(qkv_b, np.float32), np.asarray(out_w, np.float32),
        np.asarray(out_b, np.float32), np.asarray(group_scale, np.float32))
    nc = _build_program()
    kwargs = _run_kwargs or {}
    res = run_bass_kernel_spmd(nc, in_maps, core_ids=list(range(N_CORES)), **kwargs)
    out = np.empty((B, S, D), np.float32)
    for core in range(N_CORES):
        b, chunk = divmod(core, 4)
        r = res.results[core]
        yT = r["yT"] if isinstance(r, dict) else r
        out[b, chunk * CHUNK:(chunk + 1) * CHUNK] = np.asarray(yT, np.float32).T
    out += y_const
    if kwargs.get("trace"):
        kernel.last_exec_time_ns = res.exec_time_ns
    return out


if __name__ == "__main__":
    rng = np.random.default_rng(0)
    x = rng.standard_normal((B, S, D), dtype=np.float32)
    qkv_w = (rng.standard_normal((3 * D, D), dtype=np.float32) / np.sqrt(D))
    qkv_b = rng.standard_normal(3 * D, dtype=np.float32) * 0.01
    out_w = rng.standard_normal((D, D), dtype=np.float32) / np.sqrt(D)
    out_b = rng.standard_normal(D, dtype=np.float32) * 0.01
    gs = rng.standard_normal(3, dtype=np.float32)
    y = kernel(x=x, qkv_w=qkv_w, qkv_b=qkv_b, out_w=out_w, out_b=out_b,
               group_scale=gs)
    print("ok", y.shape, float(np.abs(y).mean()))
